# revision 44
# baseline (speedup 1.0000x reference)
"""Trainium2 Bass kernel for nn_Attention_73289321939579 (v4.2).

Gated attention block (AlphaFold-style):
  qkv = q_x @ w_qkv.T ; q /= sqrt(64)
  scores = q k^T + bias ; attn = softmax(scores, keys)
  o = (attn @ v) * sigmoid(q_x @ w_g.T + b_g)
  out = o @ w_o.T + b_o

Sharding over 8 cores: core = b*4 + qh*2 + hq
  b  = batch (2)            -> data parallel
  qh = query half (2x1024)  -> bias/q sliced, output row-sliced
  hq = head quad (2x4 heads)-> tensor parallel; partial outputs summed on host

Final version (~130us; v3 baseline 171us, v4.1 160us, v4.2+gate-fix 136us).
Measured-on-hw design points:
  - all PE operands bf16: fast weight load (~74ns vs 329ns fp32r), HAM
    clock stays warm (v3 spent ~100us at 1.2GHz), half the input DMA.
    numpy sim of the full bf16 pipeline: rel err 5.6e-3 (gate 2e-2).
  - O matmuls at full 128-key contraction (one MM + one PSUM bank per head
    per key chunk, lo/hi merge gone) -> PSUM: 3x s-slabs + 2x o banks.
    S stays row-tiled (T0/T8 run concurrently). Mixing (64,128)/(128,128)
    tile configs every jc measured FREE (back-to-back 215ns MMs).
  - software-pipelined emission: per jc emit S(jc)/exp(jc)/mult(jc),
    interleave piece, then O(jc-1): the next S never queues behind an O
    waiting on its pt, and a piece stalled on another engine only delays
    O (a full jc of slack), not the next exp. Steady-state exp cadence:
    median 1042ns = ACT back-to-back.
  - es/pt rings bufs=6 (bufs=4 caused a period-4 limp: every 4th exp
    waited ~650ns on the pt WAR chain).
  - minimal solid prologue (K(0,0..1), V(0..7), Q(0,0)) -> exp stream
    starts ~25us; remaining projections + the whole ic=0 epilogue ride
    inside the attention blocks (s bufs=3 keeps the exp pipeline two-deep
    despite the extra slab user; DVE per-block load kept under ACT's
    16.7us exp budget by spreading pieces across all four blocks).
  - epilogue: broadcast matmul full-128 (sel zeros kill rows 0:64),
    output projection at 128-e contraction, ocp/og_stg bf16 relocate DMAs
    split per head-pair and hidden under other pieces, bf16 output DMA in
    [P,2,512] halves (contiguous 4KB/partition descriptors).
Rejected by measurement: gpsimd eb-mult/recr offload (strict-FIFO gpsimd
queue delays whatever is last; 2.4us/slab multiplies stall the in-order PE
queue), column-tiled V projections (slower), interleaving projection
chains before S (steals the s-slab double-buffer -> one-deep exp pipeline).
"""

import sys

for _p in ("/opt/trn_rl_repo",):
    if _p not in sys.path:
        sys.path.insert(0, _p)

import numpy as np
import ml_dtypes

import concourse.bass as bass  # noqa: F401
import concourse.mybir as mybir
import concourse.tile as tile
from concourse import bacc
from concourse.bass_utils import run_bass_kernel_spmd

# ---- problem dims (hardcoded per contest contract) ----
B, Q, CQ = 2, 2048, 512
H, D = 8, 64
P = 128
QL = 1024          # queries per core
EL = 256           # e-dims per core (4 heads x 64)
HL = 4             # heads per core
CC = CQ // P       # 4 contraction chunks over channels
EC = EL // P       # 2 head-pairs
NJ = Q // P        # 16 key chunks
NI = QL // 512     # 2 query chunks of 512

F32 = mybir.dt.float32
F32R = mybir.dt.float32r
BF16 = mybir.dt.bfloat16
MUL = mybir.AluOpType.mult
ADD = mybir.AluOpType.add
EXP = mybir.ActivationFunctionType.Exp
TANH = mybir.ActivationFunctionType.Tanh

# wt_in quarter order (DMA/consumption order): K, V, Q, G
OFF_K, OFF_V, OFF_Q, OFF_G = 0, EL, 2 * EL, 3 * EL
QUARTER_OFF = [OFF_K, OFF_V, OFF_Q, OFF_G]

NWARM = 6  # HAM warmup matmuls on the memset tile


def _emit(tc, xt, ebt, wt, wot, bg, sel_in, outp):
    nc = tc.nc
    from contextlib import ExitStack

    with ExitStack() as ctx:
        const = ctx.enter_context(tc.tile_pool(name="const", bufs=1))
        esp = ctx.enter_context(tc.tile_pool(name="esp", bufs=2))
        workp = ctx.enter_context(tc.tile_pool(name="workp", bufs=2))
        odp = ctx.enter_context(tc.tile_pool(name="odp", bufs=4))
        psum = ctx.enter_context(tc.tile_pool(name="psum", bufs=2, space="PSUM"))

        # ---- warm tile first: PE busy from ~0.5us, no DMA dependency ----
        wm_sb = const.tile([P, 512], BF16, name="wm_sb", tag="wm")
        nc.vector.memset(wm_sb, 0.0)

        # ---- small constants (DMAs issued after the bulk inputs: first
        # consumers are the gate tanh / epilogue bc at ~55us) ----
        sel_sb = const.tile([P, P], F32R, name="sel_sb", tag="sel_sb")
        bg_sb = const.tile([P, EC], F32, name="bg_sb", tag="bg_sb")

        # ---- resident tensors ----
        wT_sb = const.tile([P, CC, 4 * EL], BF16, name="wT_sb", tag="wT_sb")
        xT_sb = const.tile([P, CC, Q], BF16, name="xT_sb", tag="xT_sb")
        kT_sb = const.tile([P, EC, Q], BF16, name="kT_sb", tag="kT_sb")
        q_sb = const.tile([P, EC, QL], BF16, name="q_sb", tag="q_sb")
        gp_sb = const.tile([P, EC, QL], BF16, name="gp_sb", tag="gp_sb")
        og_sb = const.tile([P, EC, QL], BF16, name="og_sb", tag="og_sb")
        woT_sb = const.tile([P, EC, CQ], BF16, name="woT_sb", tag="woT_sb")
        # V augmented with a ones column: [keychunk-part, jc, head, 65]
        va_sb = const.tile([P, NJ, HL, D + 1], BF16, name="va_sb", tag="va_sb")
        nc.vector.memset(va_sb[:, :, :, D], 1.0)
        eb_sbs = [
            const.tile([P, NJ, 512], BF16, name=f"eb{ic}", tag=f"eb{ic}")
            for ic in range(NI)
        ]
        # prezeroed fp32r reciprocal tiles (rows 65:128 must be zero for the
        # full-128 broadcast matmul; rows 0:64 are killed by sel zeros)
        recr_sbs = []
        for ri in range(2):
            rr = const.tile([P, 512], F32R, name=f"recr{ri}", tag=f"recr{ri}")
            nc.vector.memset(rr.bitcast(F32), 0.0)
            recr_sbs.append(rr)

        # ---- bulk input DMAs, consumption order, partition-first layouts ----
        def dma_w(q4):
            off = QUARTER_OFF[q4]
            nc.sync.dma_start(wT_sb[:, :, off : off + EL], wt[q4])

        def dma_x(j4):
            nc.sync.dma_start(xT_sb[:, :, j4 * 512 : (j4 + 1) * 512], xt[j4])

        dma_w(0)                                   # wk
        dma_x(0)
        dma_w(1)                                   # wv
        dma_x(1)
        dma_w(2)                                   # wq
        dma_x(2)
        dma_x(3)
        dma_w(3)                                   # wg
        nc.sync.dma_start(eb_sbs[0], ebt[0])
        nc.sync.dma_start(bg_sb, bg)
        nc.sync.dma_start(sel_sb, sel_in)
        nc.sync.dma_start(eb_sbs[1], ebt[1])
        nc.sync.dma_start(woT_sb, wot)

        # ---- HAM warmup: single stationary load, chained matmuls ----
        warm_ps = psum.tile([P, 2, 512], F32, tag="s", name="warm_ps", bufs=3)
        for wi in range(NWARM):
            nc.tensor.matmul(
                warm_ps[:, 0, :], wm_sb[:, 0:P], wm_sb,
                start=(wi == 0), stop=(wi == NWARM - 1),
            )
        warm_out = workp.tile([P, 512], F32, name="warm_out", tag="recf")
        nc.vector.tensor_copy(out=warm_out, in_=warm_ps[:, 0, :])

        # ================= projections =================
        # Chains are split into two half-thunks (2 MMs each) so interleaved
        # emission costs <=427ns of PE per attention jc. `open_ps` holds the
        # chain's psum slab across the two halves.
        open_ps = {}

        def k_half(ec, j4, half, on_act=False):
            sl = slice(j4 * 512, (j4 + 1) * 512)
            key = ("k", ec, j4)
            if half == 0:
                open_ps[key] = psum.tile([P, 2, 512], F32, tag="s",
                                         name="ps_k", bufs=3)
            ps = open_ps[key]
            for c in (0, 1) if half == 0 else (2, 3):
                nc.tensor.matmul(
                    ps[:, 0, :],
                    wT_sb[:, c, OFF_K + ec * P : OFF_K + (ec + 1) * P],
                    xT_sb[:, c, sl],
                    start=(c == 0), stop=(c == CC - 1),
                )
            if half == 1:
                if on_act:
                    nc.scalar.copy(kT_sb[:, ec, sl], ps[:, 0, :])
                else:
                    nc.vector.tensor_copy(out=kT_sb[:, ec, sl], in_=ps[:, 0, :])
                del open_ps[key]

        def q_half(ec, icc, half, on_act=False):
            sl = slice(icc * 512, (icc + 1) * 512)
            key = ("q", ec, icc)
            if half == 0:
                open_ps[key] = psum.tile([P, 2, 512], F32, tag="s",
                                         name="ps_q", bufs=3)
            ps = open_ps[key]
            for c in (0, 1) if half == 0 else (2, 3):
                nc.tensor.matmul(
                    ps[:, 0, :],
                    wT_sb[:, c, OFF_Q + ec * P : OFF_Q + (ec + 1) * P],
                    xT_sb[:, c, sl],
                    start=(c == 0), stop=(c == CC - 1),
                )
            if half == 1:
                if on_act:
                    nc.scalar.copy(q_sb[:, ec, sl], ps[:, 0, :])
                else:
                    nc.vector.tensor_copy(out=q_sb[:, ec, sl], in_=ps[:, 0, :])
                del open_ps[key]

        def g_half(ec, icc, half):
            # sigmoid(x wg^T + bg) = 0.5*tanh(0.5 x wg^T + 0.5 bg) + 0.5
            sl = slice(icc * 512, (icc + 1) * 512)
            key = ("g", ec, icc)
            if half == 0:
                open_ps[key] = psum.tile([P, 2, 512], F32, tag="s",
                                         name="ps_g", bufs=3)
            ps = open_ps[key]
            for c in (0, 1) if half == 0 else (2, 3):
                nc.tensor.matmul(
                    ps[:, 0, :],
                    wT_sb[:, c, OFF_G + ec * P : OFF_G + (ec + 1) * P],
                    xT_sb[:, c, sl],
                    start=(c == 0), stop=(c == CC - 1),
                )
            if half == 1:
                nc.scalar.activation(
                    gp_sb[:, ec, sl], ps[:, 0, :], TANH,
                    bias=bg_sb[:, ec : ec + 1], scale=0.5,
                )
                # finish the sigmoid on the idle gpsimd (gate is read by the
                # epilogue many jc later)
                nc.gpsimd.tensor_scalar(
                    gp_sb[:, ec, sl], gp_sb[:, ec, sl], 0.5, 0.5, MUL, ADD
                )
                del open_ps[key]

        def emit_v(jc):
            ps = psum.tile([P, 2, 512], F32, tag="s", name="ps_v", bufs=3)
            for c in range(CC):
                nc.tensor.matmul(
                    ps[:, 0, 0:EL],
                    xT_sb[:, c, jc * P : (jc + 1) * P],
                    wT_sb[:, c, OFF_V : OFF_V + EL],
                    start=(c == 0), stop=(c == CC - 1),
                )
            src = ps[:, 0, 0:EL].rearrange("p (h d) -> p h d", h=HL)
            nc.vector.tensor_copy(out=va_sb[:, jc, :, 0:D], in_=src)

        # solid prologue: just what attention block (0,0) needs for jc 0..7
        # (evacuations on ACT -- it is idle until the exp stream starts);
        # V(8..15) and the remaining K/Q chains ride inside the blocks
        k_half(0, 0, 0); k_half(0, 0, 1, on_act=True)
        k_half(0, 1, 0); k_half(0, 1, 1, on_act=True)
        for jc in range(6):
            emit_v(jc)
        q_half(0, 0, 0); q_half(0, 0, 1, on_act=True)

        # ================= attention =================
        o_sb_all = {}  # (ic, h) -> [65, 512] f32 numerator + denominator row

        def emit_attention(ic, hp, tail=False, interleave=None,
                           carry_in=None):
            """16 key chunks for query chunk ic, head pair hp; O matmuls run
            at full 128-key contraction one jc behind S/exp/mult. The final
            O pair + o staging are CARRIED into the next block's first two
            iterations (returned as closures): a late mult(15) then never
            serializes ahead of the next block's S in the in-order queues."""
            isl = slice(ic * 512, (ic + 1) * 512)
            ebt_sb = eb_sbs[ic]
            o_ps = [
                psum.tile([P, 512], F32, tag="o", name=f"o_ps{ic}_{hp}_{i}",
                          bufs=2)
                for i in range(2)
            ]
            pts = {}

            def emit_o(jc):
                pt = pts.pop(jc)
                for hh in range(2):
                    nc.tensor.matmul(
                        o_ps[hh][0:65, :],
                        va_sb[:, jc, 2 * hp + hh, :],
                        pt[:, hh, :],
                        start=(jc == 0), stop=(jc == NJ - 1),
                    )

            for jc in range(NJ):
                s_ps = psum.tile([P, 2, 512], F32, tag="s", name="s_ps",
                                 bufs=3)
                nc.tensor.matmul(
                    s_ps[:, 0, :],
                    kT_sb[0:64, hp, jc * P : (jc + 1) * P],
                    q_sb[0:64, hp, isl],
                    start=True, stop=True,
                )
                nc.tensor.matmul(
                    s_ps[:, 1, :],
                    kT_sb[64:128, hp, jc * P : (jc + 1) * P],
                    q_sb[64:128, hp, isl],
                    start=True, stop=True,
                )
                es_sb = esp.tile([P, 2, 512], BF16, name="es_sb", tag="es",
                                 bufs=8)
                nc.scalar.activation(
                    es_sb.rearrange("p a b -> p (a b)"),
                    s_ps.rearrange("p a b -> p (a b)"), EXP,
                )
                pt_sb = esp.tile([P, 2, 512], BF16, name="pt_sb", tag="pt",
                                 bufs=8)
                ebb = ebt_sb[:, jc, None, :].to_broadcast([P, 2, 512])
                nc.vector.tensor_tensor(pt_sb, es_sb, ebb, MUL)
                pts[jc] = pt_sb
                # interleave AFTER S/exp/mult: a piece stalled on another
                # engine then only delays O (which has a full jc of slack),
                # not the next exp
                if jc == 0 and carry_in is not None:
                    carry_in[0]()
                if jc == 1 and carry_in is not None:
                    carry_in[1]()
                if interleave is not None and jc < len(interleave):
                    interleave[jc]()
                if jc > 0:
                    emit_o(jc - 1)

            def flush_o():
                emit_o(NJ - 1)

            def flush_staging():
                for hh in range(2):
                    h = 2 * hp + hh
                    osb = odp.tile([65, 512], F32, name=f"o_sb{ic}_{h}",
                                   tag="od", bufs=6)
                    if tail and hh == 0:
                        nc.scalar.copy(osb, o_ps[hh][0:65, :])
                    else:
                        nc.vector.tensor_copy(out=osb, in_=o_ps[hh][0:65, :])
                    o_sb_all[(ic, h)] = osb

            if tail:
                flush_o()
                flush_staging()
                return None
            return [flush_o, flush_staging]

        outr = outp  # [NI, P, 4, CQ] pre-tiled output

        epi_stg = {}

        def emit_epilogue_head(ic, h, tail=False):
            """normalize one head of query chunk ic."""
            if ic not in epi_stg:
                epi_stg[ic] = (
                    workp.tile([64, EC, 512], BF16, name="og_stg", tag="ogstg"),
                    workp.tile([P, EC, 512], BF16, name="ocp_sb", tag="ocp"),
                )
            og_stg, ocp_sb = epi_stg[ic]
            hp, hh = h // 2, h % 2
            o_sb = o_sb_all[(ic, h)]
            recf_sb = workp.tile([P, 512], F32, name="recf_sb", tag="recf")
            # approx recip over the whole [65,512] accumulator; row 64 holds
            # the softmax denominators
            nc.vector.reciprocal_approx_fast(out=recf_sb[0:65, :], in_=o_sb)
            # fp32r rounding copy for the matmul operand
            recr_sb = recr_sbs[h % 2]
            if tail:
                nc.scalar.copy(recr_sb[0:65, :], recf_sb[0:65, :])
            else:
                nc.vector.tensor_copy(out=recr_sb[0:65, :], in_=recf_sb[0:65, :])
            # broadcast row 64 to all partitions (full-128 matmul: sel rows
            # other than 64 are zero, so recr rows 0:64 are killed)
            bc_ps = psum.tile([P, 2, 512], F32, tag="s", name="bc_ps", bufs=3)
            nc.tensor.matmul(
                bc_ps[:, 0, :], sel_sb, recr_sb,
                start=True, stop=True,
            )
            oc_dst = ocp_sb[0:64, hp, :] if hh == 0 else og_stg[:, hp, :]
            nc.vector.tensor_tensor(
                oc_dst, bc_ps[0:64, 0, :], o_sb[0:64, :], MUL
            )

        out_sbs = {}

        def emit_og_reloc(ic, hp):
            """relocate the odd head of pair hp to partitions 64:128."""
            og_stg, ocp_sb = epi_stg[ic]
            nc.sync.dma_start(ocp_sb[64:128, hp, :], og_stg[:, hp, :])

        def emit_og_mult(ic, hp):
            """apply the gate for head pair hp of query chunk ic."""
            isl = slice(ic * 512, (ic + 1) * 512)
            og_stg, ocp_sb = epi_stg[ic]
            nc.vector.tensor_tensor(
                og_sb[:, hp, isl], ocp_sb[:, hp, :], gp_sb[:, hp, isl], MUL
            )
            if ic not in out_sbs:
                out_sbs[ic] = odp.tile(
                    [P, 4, CQ], BF16, name="out_sb", tag="outsb", bufs=2
                )

        def emit_epilogue_ip(ic, ip4, tail=False):
            """one 128-query chunk of the output projection at full 128-e
            contraction (2 chained MMs over the ec halves)."""
            out_sb = out_sbs[ic]
            ip = ic * 4 + ip4
            slab = psum.tile([P, 2, 512], F32, tag="s", name="ps_o", bufs=3)
            for ec in range(EC):
                nc.tensor.matmul(
                    slab[:, 0, :],
                    og_sb[:, ec, ip * P : (ip + 1) * P],
                    woT_sb[:, ec, :],
                    start=(ec == 0), stop=(ec == EC - 1),
                )
            if tail:
                nc.scalar.copy(out_sb[:, ip4, :], slab[:, 0, :])
            else:
                nc.vector.tensor_copy(out=out_sb[:, ip4, :], in_=slab[:, 0, :])
            if ip4 % 2 == 1:
                # [P,2,512] halves: 4KB contiguous per partition per DMA
                nc.sync.dma_start(
                    outr[ic, :, ip4 - 1 : ip4 + 1, :],
                    out_sb[:, ip4 - 1 : ip4 + 1, :],
                )

        # interleave thunks: remaining projections and the ic=0 epilogue ride
        # inside the attention blocks (paced so every consumer is ready)
        il_00 = [
            lambda: emit_v(6),
            lambda: emit_v(7),
            lambda: (k_half(0, 2, 0), k_half(0, 2, 1)),
            lambda: emit_v(8),
            lambda: emit_v(9),
            lambda: (k_half(0, 3, 0), k_half(0, 3, 1)),
            lambda: emit_v(10),
            lambda: emit_v(11),
            lambda: emit_v(12),
            lambda: emit_v(13),
            lambda: (q_half(1, 0, 0), q_half(1, 0, 1)),
            lambda: emit_v(14),
            lambda: emit_v(15),
            lambda: (k_half(1, 0, 0), k_half(1, 0, 1)),
            lambda: (k_half(1, 1, 0), k_half(1, 1, 1)),
        ]
        il_01 = [
            lambda: k_half(1, 2, 0), lambda: k_half(1, 2, 1),
            lambda: q_half(0, 1, 0), lambda: q_half(0, 1, 1),
            lambda: g_half(0, 0, 0), lambda: g_half(0, 0, 1),
            lambda: k_half(1, 3, 0), lambda: k_half(1, 3, 1),
            lambda: q_half(1, 1, 0), lambda: q_half(1, 1, 1),
            lambda: emit_epilogue_head(0, 0),
        ]
        il_10 = [
            lambda: emit_epilogue_head(0, 1),
            lambda: emit_og_reloc(0, 0),
            lambda: g_half(1, 0, 0), lambda: g_half(1, 0, 1),
            lambda: emit_og_mult(0, 0),
            lambda: emit_epilogue_head(0, 2),
            lambda: emit_epilogue_head(0, 3),
            lambda: emit_og_reloc(0, 1),
            lambda: g_half(0, 1, 0), lambda: g_half(0, 1, 1),
            lambda: emit_og_mult(0, 1),
        ]
        il_11 = [
            lambda: emit_epilogue_ip(0, 0),
            lambda: emit_epilogue_head(1, 0),
            lambda: emit_epilogue_ip(0, 1),
            lambda: emit_epilogue_head(1, 1),
            lambda: emit_epilogue_ip(0, 2),
            lambda: emit_og_reloc(1, 0),
            lambda: emit_epilogue_ip(0, 3),
            lambda: emit_og_mult(1, 0),
            lambda: g_half(1, 1, 0), lambda: g_half(1, 1, 1),
        ]

        cry = emit_attention(0, 0, interleave=il_00)
        cry = emit_attention(0, 1, interleave=il_01, carry_in=cry)
        cry = emit_attention(1, 0, interleave=il_10, carry_in=cry)
        emit_attention(1, 1, tail=True, interleave=il_11, carry_in=cry)
        # tail order hides the odd-head relocate DMA under h(1,2)'s chain
        emit_epilogue_head(1, 3, tail=True)
        emit_og_reloc(1, 1)
        emit_epilogue_head(1, 2)  # DVE recr: runs beside h(1,3)'s ACT chain
        emit_og_mult(1, 1)
        for ip4 in range(4):
            emit_epilogue_ip(1, ip4, tail=True)


_CACHE = {}


def _get_nc():
    if "nc" not in _CACHE:
        nc = bacc.Bacc("TRN2", debug=False, enable_asserts=False)
        xt = nc.dram_tensor(
            "xt_in", [Q // 512, P, CC, 512], BF16, kind="ExternalInput"
        ).ap()
        ebt = nc.dram_tensor(
            "eb_in", [NI, P, NJ * 512], BF16, kind="ExternalInput"
        ).ap()
        wt = nc.dram_tensor(
            "wt_in", [4, P, CC, EL], BF16, kind="ExternalInput"
        ).ap()
        wot = nc.dram_tensor(
            "wot_in", [P, EC, CQ], BF16, kind="ExternalInput"
        ).ap()
        bg = nc.dram_tensor("bg_in", [P, EC], F32, kind="ExternalInput").ap()
        sel_in = nc.dram_tensor("sel_in", [P, P], F32R, kind="ExternalInput").ap()
        outp = nc.dram_tensor(
            "out", [NI, P, 4, CQ], BF16, kind="ExternalOutput"
        ).ap()
        with tile.TileContext(nc) as tc:
            _emit(tc, xt, ebt, wt, wot, bg, sel_in, outp)
        nc.compile()
        _CACHE["nc"] = nc
    return _CACHE["nc"]


LAST_RESULTS = None


def kernel(q_x, kv_x, bias, w_qkv, w_o, b_o, w_g, b_g):
    global LAST_RESULTS
    q_x = np.asarray(q_x, np.float32)
    bias = np.asarray(bias, np.float32)
    w_qkv = np.asarray(w_qkv, np.float32)
    w_o = np.asarray(w_o, np.float32)
    b_o = np.asarray(b_o, np.float32)
    w_g = np.asarray(w_g, np.float32)
    b_g = np.asarray(b_g, np.float32)
    BH = ml_dtypes.bfloat16

    # selection matrix: row 64 ones (reciprocal broadcast)
    sel = np.zeros((P, P), np.float32)
    sel[64, :] = 1.0
    in_maps = []
    for core in range(8):
        b, qh, hq = core >> 2, (core >> 1) & 1, core & 1
        i0 = qh * QL
        esl = slice(hq * EL, (hq + 1) * EL)
        xTb = q_x[b].T  # [512, 2048]
        # roll keys so this core's queries are columns 0:QL
        xTp = np.concatenate([xTb[:, i0:], xTb[:, :i0]], axis=1)
        # pre-tile: [j4, p, c, 512] with rows = chans c*128+p
        xtp = np.ascontiguousarray(
            xTp.reshape(CC, P, 4, 512).transpose(2, 1, 0, 3)
        ).astype(BH)
        biasTb = bias[b, 0].T  # [keys, queries]
        ebp = np.exp(
            np.concatenate(
                [biasTb[i0:, i0 : i0 + QL], biasTb[:i0, i0 : i0 + QL]], axis=0
            )
        ).astype(BH)
        # pre-tile: [ic, p, jc*512+q], keys = jc*128 + p
        ebtp = np.ascontiguousarray(
            ebp.reshape(NJ, P, NI, 512).transpose(2, 1, 0, 3).reshape(
                NI, P, NJ * 512
            )
        )
        wq = w_qkv[0:CQ][esl] * (1.0 / np.sqrt(D))
        wk = w_qkv[CQ : 2 * CQ][esl]
        wv = w_qkv[2 * CQ : 3 * CQ][esl]
        wg = w_g[esl]
        # pre-tile: [quarter(K,V,Q,G), p, c, e] with rows = chans c*128+p
        wtp = np.ascontiguousarray(
            np.stack(
                [
                    w.T.reshape(CC, P, EL).transpose(1, 0, 2)
                    for w in (wk, wv, wq, wg)
                ],
                axis=0,
            )
        ).astype(BH)
        # woT pre-tiled: [p, o, c] with e-dim = o*128+p
        woTc = w_o[:, esl].T  # [256, 512]
        wotp = np.ascontiguousarray(
            woTc.reshape(EC, P, CQ).transpose(1, 0, 2)
        ).astype(BH)
        bgc = np.ascontiguousarray(
            (0.5 * b_g[esl]).reshape(EC, P).T, np.float32
        )
        in_maps.append(
            {
                "xt_in": xtp,
                "eb_in": ebtp,
                "wt_in": wtp,
                "wot_in": wotp,
                "bg_in": bgc,
                "sel_in": sel,
            }
        )

    nc = _get_nc()
    res = run_bass_kernel_spmd(nc, in_maps, core_ids=list(range(8)))
    LAST_RESULTS = res

    out = np.zeros((B, Q, CQ), np.float32)
    for core in range(8):
        b, qh = core >> 2, (core >> 1) & 1
        i0 = qh * QL
        # out tensor is [NI, P, 4, CQ]: q row = ic*512 + o*128 + p
        arr = res.results[core]["out"].astype(np.float32)
        out[b, i0 : i0 + QL] += arr.transpose(0, 2, 1, 3).reshape(QL, CQ)
    out += b_o
    return out
